# revision 24
# baseline (speedup 1.0000x reference)
"""Trainium2 Bass kernel for single-head full-softmax attention.

Reference computation (B=4, T=4096, D=768, H=64):
    Q = x @ Wq.T + bq ; K = x @ Wk.T + bk ; V = x @ Wv.T + bv
    out = softmax(Q K^T / 8) @ V          (no causal mask)

Sharding: 8 cores; core i owns batch b=i//2, query half i%2 (2048 queries).
Each core projects Q/K/V for its own 2048 tokens; K/V halves are
exchanged within core pairs {2b, 2b+1} via AllGather, and each core runs
attention for its 2048 queries against the full 4096 keys.  K/V tiles
are laid out LOCAL-first: the own half comes straight from the
projection (no DRAM round trip), the partner half is pulled from the
AllGather result with a dynamic-offset DMA (host passes the partner
section index per core), so local attention overlaps the collective.

Host-side prep (pure layout transforms, all FLOPs stay on device):
  - x shard transposed to x^T [768, 2048] bf16 (d-contraction on SBUF
    partitions, no on-chip transpose).
  - Wq/Wk pre-transposed AND column-duplicated to [768, 128] so the
    projections materialize Q^T/K^T on both partition halves — enables
    row-group-packed QK^T matmuls (two k-tiles run concurrently in the
    128x128 PE array since the contraction dim is only 64).
  - Wv gets a zero 65th column and bv a 1.0 65th element so V1 = [V | 1]
    comes out of the projection directly: P @ V1 yields numerator and
    softmax denominator in one PSUM accumulation.
  - bk dropped: it shifts each query row's scores by a constant, which
    softmax cancels exactly.
  - all weights packed into two DMAs (one bf16, one f32).

On-chip dataflow per core (matmuls bf16, PSUM fp32):
  scores transposed per k-tile: S^T[k,q] = matmul(lhsT=K^T tile, rhs=Q^T);
  exp on ScalarE with scale=0.125 folded in (scores are O(1) — no max
  pass); P^T bf16; out^T[h1,q] += V1[kt].T @ P^T[kt] over 32 k-tiles.
  Tail per query chunk: PE-transpose out^T to [q, 65], reciprocal of the
  denominator column, per-partition scalar multiply, one output DMA.
"""

import numpy as np
import ml_dtypes

import concourse.bass as bass
import concourse.tile as tile
from concourse import bacc, mybir
from concourse.bass import ts, ds
from concourse.bass_utils import run_bass_kernel_spmd
from concourse.masks import make_identity

BF16 = mybir.dt.bfloat16
F32 = mybir.dt.float32

B, T, D, H = 4, 4096, 768, 64
H1 = H + 1          # V augmented with ones column
NCORES = 8
TL = T // 2         # 2048 local tokens / queries per core
DT = D // 128       # 6 d-tiles
KT = T // 128       # 32 k-tiles over the full sequence
KTL = TL // 128     # 16 k-tiles per half
QC = TL // 512      # 4 query chunks of 512
SCALE = 1.0 / 8.0   # 1/sqrt(64)
WCOLS = 128 + 128 + H1   # packed weight columns (wq2 | wk2 | wv1)

K_ELEMS = H * TL          # 64*2048   bf16 elements of K^T payload
V_ELEMS = TL * H1         # 2048*65   bf16 elements of V1 payload
KV_ELEMS = K_ELEMS + V_ELEMS

REPLICA_GROUPS = [[0, 1], [2, 3], [4, 5], [6, 7]]
EXP = mybir.ActivationFunctionType.Exp
IDENT = mybir.ActivationFunctionType.Identity


def build_body(nc, tc, ap, psum, sbuf, fake_collective=False):
    """Emit one full forward pass. ap: dict of DRAM APs."""

    # ---- x^T pieces in column-chunk-major order: chunk c of K/Q proj
    # only needs columns ts(c,512), so the first projections start ~4us in
    xT_sb = sbuf.tile([128, DT, TL], BF16, tag="xT", bufs=1)
    def emit_xt_piece(c, d):
        nc.sync.dma_start(out=xT_sb[:, d, ts(c, 512)],
                          in_=ap["xT"][ds(d * 128, 128), ts(c, 512)])
    for d in range(DT):
        emit_xt_piece(0, d)

    # ---- packed weights on the SWDGE queue (parallel with x^T DMAs) ----
    wpack_sb = sbuf.tile([128, DT, WCOLS], BF16, tag="wpack", bufs=1)
    bpack_sb = sbuf.tile([128, 1 + H1], F32, tag="bpack", bufs=1)
    nc.scalar.dma_start(
        out=wpack_sb, in_=ap["wpack"].rearrange("(i p) h -> p i h", p=128))
    nc.scalar.dma_start(out=bpack_sb, in_=ap["bpack"])
    wq_sb = wpack_sb[:, :, 0:128]
    wk_sb = wpack_sb[:, :, 128:256]
    wv_sb = wpack_sb[:, :, 256:WCOLS]
    bq_sb = bpack_sb[:, 0:1]
    bv1_sb = bpack_sb[:, 1:1 + H1]

    for c in range(1, QC):
        for d in range(DT):
            emit_xt_piece(c, d)

    ident = sbuf.tile([128, 128], F32, tag="ident", bufs=1)
    make_identity(nc, ident)

    # PE warm-up during the initial DMA wait: the HAM clock gate runs the
    # array at 1.2 GHz until ~3.4us of sustained activity; burn idle time
    # on throwaway matmuls so the real projections run at 2.4 GHz.
    warm_sb = sbuf.tile([128, 64], BF16, tag="warm", bufs=1)
    nc.vector.memset(warm_sb, 0.0)
    warm_ps = psum.tile([64, 64], F32, tag="o", name="warm_ps")
    for _ in range(40):
        nc.tensor.matmul(warm_ps, warm_sb[:, 0:64], warm_sb[:, 0:64],
                         start=True, stop=True)

    # ---- K^T/V1/Q^T: local tiles land directly in the attention buffers
    k2_sb = sbuf.tile([128, T], BF16, tag="k", bufs=1)      # row-duplicated
    v1_sb = sbuf.tile([128, KT, H1], BF16, tag="v1", bufs=1)
    q2_sb = sbuf.tile([128, TL], BF16, tag="q", bufs=1)     # row-duplicated

    # DRAM bounce buffers for the pair exchange (emitted mid-attention)
    dram_cm = tc.tile_pool(name="dram", bufs=1, space="DRAM")
    dram = dram_cm.__enter__()
    bounce_in = dram.tile([KV_ELEMS], BF16)
    bounce_out = dram.tile([2, KV_ELEMS], BF16)

    def emit_v_tile(t):
        vp_ps = psum.tile([128, H1], F32, tag="o", name="vp_ps")
        for d in range(DT):
            nc.tensor.matmul(
                vp_ps, xT_sb[:, d, ts(t, 128)], wv_sb[:, d, :],
                start=(d == 0), stop=(d == DT - 1),
            )
        nc.vector.tensor_add(v1_sb[:, t, :], vp_ps, bv1_sb)

    def emit_exchange():
        nc.sync.dma_start(
            out=bounce_in[0:K_ELEMS].rearrange("(p t) -> p t", p=H),
            in_=k2_sb[0:H, 0:TL],
        )
        nc.sync.dma_start(
            out=bounce_in[K_ELEMS:].rearrange("(t p h) -> p t h", t=KTL, p=128),
            in_=v1_sb[:, 0:KTL, :],
        )
        if fake_collective:
            nc.sync.dma_start(out=bounce_out[0], in_=bounce_in)
            nc.sync.dma_start(out=bounce_out[1], in_=bounce_in)
        else:
            nc.gpsimd.collective_compute(
                "AllGather",
                mybir.AluOpType.bypass,
                replica_groups=REPLICA_GROUPS,
                ins=[bounce_in.opt()],
                outs=[bounce_out.opt()],
            )

    def emit_q23():
        emit_q_chunk(2)
        emit_q_chunk(3)

    def emit_gather_in():
        # partner half -> k2 cols [2048:4096] / v1 tiles [16:32], via a
        # dynamic offset: psec = partner section index within the pair
        psec_reg = nc.gpsimd.alloc_register(f"psec_reg_{nc.next_id()}")
        nc.gpsimd.reg_load(psec_reg, ap["psec"][0:1, 0:1])
        psec = nc.gpsimd.snap(psec_reg, donate=True, min_val=0, max_val=1)
        for r in range(2):
            nc.gpsimd.dma_start(
                out=k2_sb[ds(r * H, H), ds(TL, TL)],
                in_=bounce_out[ds(psec, 1), 0:K_ELEMS].rearrange(
                    "s (p t) -> p (s t)", p=H),
            )
        nc.gpsimd.dma_start(
            out=v1_sb[:, ds(KTL, KTL), :],
            in_=bounce_out[ds(psec, 1), K_ELEMS:].rearrange(
                "s (t p h) -> p (s t) h", t=KTL, p=128),
        )


    def emit_k_chunk(c):
        ps = psum.tile([128, 512], F32, tag="o", name=f"kp_ps{c}")
        for d in range(DT):
            nc.tensor.matmul(ps, wk_sb[:, d, :], xT_sb[:, d, ts(c, 512)],
                             start=(d == 0), stop=(d == DT - 1))
        nc.vector.tensor_copy(out=k2_sb[:, ts(c, 512)], in_=ps)

    def emit_q_chunk(c):
        ps = psum.tile([128, 512], F32, tag="o", name=f"qp_ps{c}")
        for d in range(DT):
            nc.tensor.matmul(ps, wq_sb[:, d, :], xT_sb[:, d, ts(c, 512)],
                             start=(d == 0), stop=(d == DT - 1))
        nc.vector.tensor_scalar_add(q2_sb[:, ts(c, 512)], ps, bq_sb)

    # chunk-major, following the x^T DMA order; V tiles fill the
    # DMA-paced gaps of the projection phase
    emit_k_chunk(0)
    emit_q_chunk(0)
    for t in range(0, 4):
        emit_v_tile(t)
    emit_k_chunk(1)
    emit_q_chunk(1)
    for t in range(4, 8):
        emit_v_tile(t)
    emit_k_chunk(2)
    for t in range(8, 12):
        emit_v_tile(t)
    emit_k_chunk(3)
    for t in range(12, 16):
        emit_v_tile(t)
    emit_exchange()
    assert KTL == 16

    # ---- attention ----
    # segment order: c0-local c1-local c0-remote c1-remote c2L c2R c3L c3R
    # (locals never wait on the collective; c0R starts ~16 pairs in)
    out_stage = sbuf.tile([H1, QC, 512], F32, tag="ostage", bufs=1)
    outf_sb = sbuf.tile([128, KTL, H], F32, tag="outf", bufs=1)
    stage_flat = out_stage.rearrange("p c q -> p (c q)")

    out_dram = ap["out"].rearrange("(i p) h -> p i h", p=128)

    def emit_tail(c):
        for i in range(4):
            g = 4 * c + i
            tr_ps = psum.tile([128, H1], F32, tag="o", name="tr_ps")
            nc.tensor.transpose(tr_ps, stage_flat[:, ts(g, 128)],
                                ident[0:H1, 0:H1])
            rcp = sbuf.tile([128, 1], F32, tag="rcp", bufs=2)
            nc.vector.reciprocal(rcp, tr_ps[:, H:H1])
            nc.vector.tensor_scalar_mul(outf_sb[:, g, :], tr_ps[:, 0:H], rcp)
        nc.sync.dma_start(out=out_dram[:, ds(4 * c, 4), :],
                          in_=outf_sb[:, ds(4 * c, 4), :])

    # k-tiles grouped in pairs: one exp instruction covers FD=1024
    segments = [(0, 0), (1, 0), (0, 1), (1, 1), (2, 0), (2, 1), (3, 0), (3, 1)]
    GROUPS = [range(2 * i, 2 * i + 2) for i in range(8)]
    steps = [(c, [16 * side + k for k in g])
             for c, side in segments for g in GROUPS]

    o_ps = {}
    prev = None          # (c, kts, pt)
    pending = []         # chunks whose copy is done, tail not yet emitted

    def flush_prev():
        nonlocal prev
        if prev is None:
            return
        pc, pkts, ppt = prev
        for j, kt in enumerate(pkts):
            nc.tensor.matmul(
                o_ps[pc], v1_sb[:, kt, :], ppt[:, j],
                start=(kt == 0), stop=(kt == KT - 1),
            )
        if pkts[-1] == KT - 1:
            nc.vector.tensor_copy(out=out_stage[:, pc, :], in_=o_ps[pc])
            del o_ps[pc]
            pending.append(pc)
        prev = None

    for idx, (c, kts) in enumerate(steps):
        if c not in o_ps and kts[0] == 0:
            o_ps[c] = psum.tile([H1, 512], F32, tag="o", name=f"o_ps{c}")
        st = psum.tile([128, 2, 512], F32, tag="st", bufs=3, name="st")
        for j, kt in enumerate(kts):
            nc.tensor.matmul(
                st[:, j],
                k2_sb[ds(64 * (j % 2), 64), ts(kt, 128)],
                q2_sb[ds(64 * (j % 2), 64), ts(c, 512)],
                start=True, stop=True,
            )
        n = len(kts)
        pt = sbuf.tile([128, 2, 512], BF16, tag="pt", bufs=3)
        nc.scalar.activation(out=pt[:, 0:n], in_=st[:, 0:n],
                             func=EXP, scale=SCALE)
        flush_prev()
        prev = (c, kts, pt)
        if idx == 1:
            emit_gather_in()
        elif idx == 18:
            emit_q23()    # PE has slack here; q2/q3 needed from step 32
        if pending and kts[0] % 16 == 4:
            emit_tail(pending.pop(0))
    flush_prev()
    for c in pending:
        emit_tail(c)
    dram_cm.__exit__(None, None, None)


def build(repeat=1, fake_collective=False, num_devices=NCORES,
          timing_mode=False):
    nc = bacc.Bacc("TRN2", target_bir_lowering=False, debug=False,
                   num_devices=num_devices)
    # timing_mode: x^T becomes an Internal scratch tensor (content
    # irrelevant) so benchmark calls ship ~100KB instead of 25MB and the
    # NEFF execution dominates the wall clock.
    xT_kind = "Internal" if timing_mode else "ExternalInput"
    ap = {
        "xT": nc.dram_tensor("xT", [D, TL], BF16, kind=xT_kind).ap(),
        "wpack": nc.dram_tensor("wpack", [D, WCOLS], BF16,
                                kind="ExternalInput").ap(),
        "bpack": nc.dram_tensor("bpack", [128, 1 + H1], F32,
                                kind="ExternalInput").ap(),
        "psec": nc.dram_tensor("psec", [1, 1], mybir.dt.uint32,
                               kind="ExternalInput").ap(),
        "out": nc.dram_tensor("out", [TL, H], F32, kind="ExternalOutput").ap(),
    }
    with tile.TileContext(nc) as tc:
        with tc.tile_pool(name="psum", bufs=2, space="PSUM") as psum, \
             tc.tile_pool(name="sbuf", bufs=2) as sbuf:
            for _ in range(repeat):
                build_body(nc, tc, ap, psum, sbuf, fake_collective)
    nc.compile()
    return nc


def make_in_maps(x, Wq, bq, Wk, bk, Wv, bv):
    """Per-core input shards. bk is intentionally unused (softmax-invariant)."""
    del bk
    x = np.asarray(x, np.float32)
    wqT = np.asarray(Wq, np.float32).T                      # [768, 64]
    wkT = np.asarray(Wk, np.float32).T
    wv1 = np.concatenate(
        [np.asarray(Wv, np.float32).T, np.zeros((D, 1), np.float32)], axis=1)
    wpack = np.concatenate([wqT, wqT, wkT, wkT, wv1], axis=1)
    wpack_h = np.ascontiguousarray(wpack).astype(ml_dtypes.bfloat16)
    bq1 = np.asarray(bq, np.float32).reshape(H, 1)
    bq2 = np.concatenate([bq1, bq1], axis=0)                # [128, 1]
    bv1 = np.tile(
        np.concatenate([np.asarray(bv, np.float32), [1.0]])[None, :], (128, 1))
    bpack_h = np.ascontiguousarray(
        np.concatenate([bq2, bv1], axis=1), dtype=np.float32)

    in_maps = []
    for i in range(NCORES):
        b, half = i // 2, i % 2
        xh = x[b, half * TL:(half + 1) * TL, :]          # [2048, 768]
        xT = np.ascontiguousarray(xh.T).astype(ml_dtypes.bfloat16)
        in_maps.append({
            "xT": xT, "wpack": wpack_h, "bpack": bpack_h,
            "psec": np.array([[1 - (i % 2)]], np.uint32),
        })
    return in_maps


_NC_CACHE = {}


def kernel(x, Wq, bq, Wk, bk, Wv, bv):
    if "nc" not in _NC_CACHE:
        _NC_CACHE["nc"] = build()
    nc = _NC_CACHE["nc"]
    in_maps = make_in_maps(x, Wq, bq, Wk, bk, Wv, bv)
    res = run_bass_kernel_spmd(nc, in_maps, core_ids=list(range(NCORES)))
    out = np.empty((B, T, H), np.float32)
    for i in range(NCORES):
        b, half = i // 2, i % 2
        out[b, half * TL:(half + 1) * TL, :] = res.results[i]["out"]
    return out
